# revision 13
# baseline (speedup 1.0000x reference)
"""CenterLoss kernel for Trainium2 (8 NeuronCores, data-parallel over batch).

loss = mean_i( ||nx_i||^2 + ||c_{l_i}||^2 - 2*nx_i.c_{l_i} )
     = mean_i( ||nx_i - c_{l_i}||^2 ),  nx_i = x_i / max(||x_i||, EPS)

The reference's (batch, num_classes) distmat is masked down to one column
per row, so only a gather of centers[labels] is needed (memory regime).

Sharding: batch 16384 -> 8 cores x 2048 rows, centers replicated. x row
p*16+j of a core's shard lives at SBUF partition p, free block j
(natural contiguous DMA). The centers gather uses InstDMAGatherAnt
(gpsimd dma_gather), which is Q7 descriptor-generation bound (~8.5ns
per gathered row on one tx/rx core pair; a hot-labels probe showed no
DRAM-locality sensitivity). The ucode runs queue q's desc-gen on Q7
core pair q (cpu_id/2 == queue_num; the plain indirect-DMA path is
frozen to pair 0), so the gather is split across all 4 SWDGE queues to
parallelize desc-gen across core pairs — confirmed on HW: queue 1-3
gathers retire in ~100ns because their pairs ran ahead during queue
0's slice. The mlp library's ~6us lazy IRAM load happens at the FIRST
extended-inst dispatch, so a 128-idx warmup gather (idxs memset to 0)
issues before the label wait to pull that load under the input DMAs;
queue 0 gets fewer real rows to compensate. Gather slot i writes
dst[i % 128, i // 128]; labels are permuted host-side (slot j*128+p =
label of x row p*16+j) and pre-wrapped into the int16 [16, n/16]
layout (replicated to all 8 Q7 core groups). Each gather has its own
semaphore (a shared sem races: its value can be a mix of two gathers'
per-engine completions). A dummy sqrt preloads the ACT table under the
DMA shadow. The x-side pipeline (square/rowsum/sqrt/recip/normalize)
runs under the gathers on ACT+DVE; d = nx - c and the Square+
accumulate reduction are chunked per gather, ordered by expected
completion. Raw bacc with manual semaphores. Each core returns
per-partition partial sums; the host combines.
"""

import numpy as np

B, C, D = 16384, 8192, 64
N_CORES = 8
ROWS = B // N_CORES        # 2048
P = 128
J = ROWS // P              # 16 blocks of D per partition
F = J * D                  # 1024 f32 per partition
BLOCKS = [3, 5, 4, 4]      # J-blocks per gather/queue (pair 0 also warms up)
NGATH = len(BLOCKS)
BSTART = [sum(BLOCKS[:g]) for g in range(NGATH)]
SUB_ORDER = [0, 2, 3, 1]   # chunk issue order ~ expected completion order
WARM = 128                 # warmup gather idxs

_CACHE = {}


def _build():
    from contextlib import ExitStack

    import concourse.bass as bass
    from concourse import bacc, library_config, mybir

    nc = bacc.Bacc("TRN2", target_bir_lowering=False, debug=False,
                   num_devices=N_CORES, dynamic_dma_scratch_size=65536,
                   num_swdge_queues=NGATH)
    f32 = mybir.dt.float32
    x = nc.dram_tensor("x", [ROWS, D], f32, kind="ExternalInput").ap()
    labels = nc.dram_tensor("labels", [P, ROWS // 16], mybir.dt.int16,
                            kind="ExternalInput").ap()
    centers = nc.dram_tensor("centers", [C, D], f32,
                             kind="ExternalInput").ap()
    out = nc.dram_tensor("out", [P, NGATH], f32, kind="ExternalOutput").ap()

    with ExitStack() as ctx:
        def sb(n, s, dt=f32):
            return ctx.enter_context(nc.sbuf_tensor(n, s, dt))
        lab_t = sb("lab_t", [P, ROWS // 16], mybir.dt.int16)
        widx = sb("widx", [P, WARM // 16], mybir.dt.int16)
        warm_t = sb("warm_t", [P, D])
        x_t = sb("x_t", [P, F])
        c_t = sb("c_t", [P, F])
        xx = sb("xx", [P, F])
        sx = sb("sx", [P, J])
        mn = sb("mn", [P, J])
        inv = sb("inv", [P, J])
        nx = sb("nx", [P, F])
        acc = sb("acc", [P, NGATH])
        L = ctx.enter_context(nc.semaphore("Lsem"))
        X = ctx.enter_context(nc.semaphore("Xsem"))
        W = ctx.enter_context(nc.semaphore("Wsem"))
        M = ctx.enter_context(nc.semaphore("Msem"))
        G = [ctx.enter_context(nc.semaphore(f"Gsem{g}")) for g in range(NGATH)]
        A = ctx.enter_context(nc.semaphore("Asem"))   # ACT-produced events
        V = ctx.enter_context(nc.semaphore("Vsem"))   # DVE-produced events

        # ---- Sync: labels in, result out ----
        nc.sync.dma_start(lab_t[:], labels[:]).then_inc(L, 16)
        nc.sync.wait_ge(A, 2 + NGATH)
        nc.sync.dma_start(out, acc[:]).then_inc(L, 16)
        nc.sync.wait_ge(L, 32)

        # ---- GpSimd: warmup (pull the lazy mlp IRAM load early), gathers --
        nc.gpsimd.load_library(library_config.mlp)
        nc.gpsimd.memset(widx[:], 0).then_inc(M, 1)
        nc.gpsimd.wait_ge(M, 1)
        nc.gpsimd.dma_gather(
            warm_t[:].rearrange("p (j d) -> p j d", d=D),
            centers[:], widx[:], WARM, WARM, D, queue_num=0,
        ).then_inc(W, 16)
        nc.gpsimd.wait_ge(L, 16)
        # gather g covers slots/blocks [BSTART[g], BSTART[g]+BLOCKS[g]) on
        # SWDGE queue g (queue q's desc-gen runs on Q7 core pair q).
        for g in range(NGATH):
            gr = BLOCKS[g] * P
            nc.gpsimd.dma_gather(
                c_t[:, BSTART[g] * D:(BSTART[g] + BLOCKS[g]) * D].rearrange(
                    "p (j d) -> p j d", d=D),
                centers[:],
                lab_t[:, BSTART[g] * (P // 16):
                      (BSTART[g] + BLOCKS[g]) * (P // 16)],
                gr, gr, D, queue_num=g,
            ).then_inc(G[g], 16)
        nc.gpsimd.wait_ge(W, 16)

        # ---- Scalar/ACT: x in on its HWDGE ring, squares ----
        # A events: 1=xx, 2=mn(sqrt), 2+i+1 = i-th issued chunk accumulated
        nc.scalar.dma_start(x_t[:], x.rearrange("(p j) d -> p (j d)", p=P)
                            ).then_inc(X, 16)
        # Dummy sqrt (scale=0, bias=1 -> sqrt(1)) pulls the ACT table load
        # under the DMA shadow; mn is rewritten by the real sqrt below.
        nc.scalar.activation(mn[:, :1], mn[:, :1],
                             mybir.ActivationFunctionType.Sqrt,
                             bias=1.0, scale=0.0)
        nc.scalar.wait_ge(X, 16)
        nc.scalar.square(xx[:], x_t[:]).then_inc(A, 1)
        nc.scalar.wait_ge(V, 1)
        nc.scalar.sqrt(mn[:], sx[:]).then_inc(A, 1)
        for i, k in enumerate(SUB_ORDER):
            f0, fn = BSTART[k] * D, BLOCKS[k] * D
            nc.scalar.wait_ge(V, 4 + i)
            nc.scalar.activation(c_t[:, f0:f0 + fn], c_t[:, f0:f0 + fn],
                                 mybir.ActivationFunctionType.Square,
                                 accum_out=acc[:, k:k + 1]).then_inc(A, 1)

        # ---- Vector/DVE ----
        # V events: 1=sx, 2=inv, 3=nx, 3+i+1 = i-th issued chunk sub done
        nc.vector.wait_ge(A, 1)
        nc.vector.reduce_sum(sx[:], xx[:].rearrange("p (j d) -> p j d", d=D),
                             axis=mybir.AxisListType.X).then_inc(V, 1)
        nc.vector.wait_ge(A, 2)
        nc.vector.reciprocal(inv[:], mn[:]).then_inc(V, 1)
        nc.vector.wait_ge(V, 2)
        iap = inv[:]
        inv_bc = bass.AP(tensor=iap.tensor, offset=iap.offset,
                         ap=list(iap.ap) + [[0, D]])
        nc.vector.tensor_tensor(
            out=nx[:].rearrange("p (j d) -> p j d", d=D),
            in0=x_t[:].rearrange("p (j d) -> p j d", d=D),
            in1=inv_bc,
            op=mybir.AluOpType.mult,
        ).then_inc(V, 1)
        nc.vector.wait_ge(V, 3)
        for k in SUB_ORDER:
            f0, fn = BSTART[k] * D, BLOCKS[k] * D
            nc.vector.wait_ge(G[k], 16)
            nc.vector.tensor_sub(c_t[:, f0:f0 + fn], nx[:, f0:f0 + fn],
                                 c_t[:, f0:f0 + fn]).then_inc(V, 1)

    nc.compile()
    return nc


def _get_nc():
    if "nc" not in _CACHE:
        _CACHE["nc"] = _build()
    return _CACHE["nc"]


def _prep_labels(lab_shard):
    """int16 idx layout for dma_gather: gather slot i = j*128+p must hold
    the label of x row p*16+j (so dst[i%128, i//128] aligns with x_t);
    then wrap slots into 16 partitions (idxs[c, s] = slot s*16+c) and
    replicate for the 8 Q7 core groups."""
    slots = lab_shard.reshape(P, J).T.reshape(-1)          # slot j*128+p
    wrapped = slots.reshape(ROWS // 16, 16).T              # [16, ROWS/16]
    return np.ascontiguousarray(
        np.tile(wrapped, (8, 1)).astype(np.int16))         # [128, ROWS/16]


def _run(x, labels, centers, trace=False):
    from concourse.bass_utils import run_bass_kernel_spmd

    x = np.ascontiguousarray(np.asarray(x, dtype=np.float32))
    labels = np.asarray(labels).astype(np.int16)
    centers = np.ascontiguousarray(np.asarray(centers, dtype=np.float32))

    in_maps = []
    for i in range(N_CORES):
        in_maps.append({
            "x": x[i * ROWS:(i + 1) * ROWS],
            "labels": _prep_labels(labels[i * ROWS:(i + 1) * ROWS]),
            "centers": centers,
        })
    res = run_bass_kernel_spmd(_get_nc(), in_maps,
                               core_ids=list(range(N_CORES)), trace=trace)
    total = np.float64(0.0)
    for r in res.results:
        total += np.float64(r["out"].sum(dtype=np.float64))
    loss = np.array(np.float32(total / B))
    return loss, res


def kernel(x, labels, centers):
    loss, _ = _run(x, labels, centers, trace=False)
    return loss


# revision 15
# speedup vs baseline: 1.0526x; 1.0526x over previous
"""CenterLoss kernel for Trainium2 (8 NeuronCores, data-parallel over batch).

loss = mean_i( ||nx_i||^2 + ||c_{l_i}||^2 - 2*nx_i.c_{l_i} )
     = mean_i( ||nx_i - c_{l_i}||^2 ),  nx_i = x_i / max(||x_i||, EPS)

The reference's (batch, num_classes) distmat is masked down to one column
per row, so only a gather of centers[labels] is needed (memory regime).

Sharding: batch 16384 -> 8 cores x 2048 rows, centers replicated. x row
p*16+j of a core's shard lives at SBUF partition p, free block j.
The centers gather uses InstDMAGatherAnt (gpsimd dma_gather), which is
Q7 descriptor-generation bound (~8.5ns/row on one tx/rx core pair; a
hot-labels probe showed no DRAM-locality sensitivity). Queue q's
desc-gen runs on Q7 core pair q (the plain indirect-DMA path is frozen
to pair 0), so the gather is split across all 4 SWDGE queues; HW
confirmed the pairs run ahead and overlap. The mlp library's ~6us IRAM
load is triggered by the FIRST Q7-executed op after the reload and
contends with concurrent HBM DMAs (10us when racing the x load), so a
tiny memset right after the reload triggers it immediately and signals
T; the x DMA waits for T and is split across the sync+scalar HWDGE
rings with the normalize pipeline run per half. Gather slot i writes
dst[i%128, i//128]; labels are permuted host-side (slot j*128+p =
label of x row p*16+j) and pre-wrapped into the int16 [16, n/16]
layout (replicated to all 8 Q7 core groups). Each gather has its own
semaphore (a shared sem races: its value can mix two gathers'
per-engine completions). A dummy sqrt preloads the ACT table. d = nx-c
and the Square+accumulate reduction are chunked per gather. Raw bacc
with manual semaphores. Each core returns per-partition partial sums;
the host combines.
"""

import numpy as np

B, C, D = 16384, 8192, 64
N_CORES = 8
ROWS = B // N_CORES        # 2048
P = 128
J = ROWS // P              # 16 blocks of D per partition
F = J * D                  # 1024 f32 per partition
NGATH = 4                  # one gather per SWDGE queue / Q7 pair
GBLK = J // NGATH          # 4 J-blocks per gather
H = F // 2                 # x pipeline half size

_CACHE = {}


def _build():
    from contextlib import ExitStack

    import concourse.bass as bass
    from concourse import bacc, library_config, mybir

    nc = bacc.Bacc("TRN2", target_bir_lowering=False, debug=False,
                   num_devices=N_CORES, dynamic_dma_scratch_size=131072,
                   num_swdge_queues=NGATH)
    f32 = mybir.dt.float32
    x = nc.dram_tensor("x", [ROWS, D], f32, kind="ExternalInput").ap()
    labels = nc.dram_tensor("labels", [P, ROWS // 16], mybir.dt.int16,
                            kind="ExternalInput").ap()
    centers = nc.dram_tensor("centers", [C, D], f32,
                             kind="ExternalInput").ap()
    out = nc.dram_tensor("out", [P, NGATH], f32, kind="ExternalOutput").ap()

    with ExitStack() as ctx:
        def sb(n, s, dt=f32):
            return ctx.enter_context(nc.sbuf_tensor(n, s, dt))
        lab_t = sb("lab_t", [P, ROWS // 16], mybir.dt.int16)
        tw = sb("tw", [P, 1])
        x_t = sb("x_t", [P, F])
        c_t = sb("c_t", [P, F])
        xx = sb("xx", [P, F])
        sx = sb("sx", [P, J])
        mn = sb("mn", [P, J])
        inv = sb("inv", [P, J])
        nx = sb("nx", [P, F])
        acc = sb("acc", [P, NGATH])
        L = ctx.enter_context(nc.semaphore("Lsem"))
        T = ctx.enter_context(nc.semaphore("Tsem"))
        X1 = ctx.enter_context(nc.semaphore("X1sem"))
        X2 = ctx.enter_context(nc.semaphore("X2sem"))
        G = [ctx.enter_context(nc.semaphore(f"Gsem{g}")) for g in range(NGATH)]
        A = ctx.enter_context(nc.semaphore("Asem"))   # ACT-produced events
        V = ctx.enter_context(nc.semaphore("Vsem"))   # DVE-produced events

        xr = x.rearrange("(p j) d -> p (j d)", p=P)

        # ---- Sync: labels in, x half 2 (after IRAM load), result out ----
        nc.sync.dma_start(lab_t[:], labels[:]).then_inc(L, 16)
        nc.sync.wait_ge(T, 1)
        nc.sync.dma_start(x_t[:, H:], xr[:, H:]).then_inc(X2, 16)
        nc.sync.wait_ge(A, 4 + NGATH)
        nc.sync.dma_start(out, acc[:]).then_inc(L, 16)
        nc.sync.wait_ge(L, 32)

        # ---- GpSimd: trigger the mlp IRAM load, then the gathers ----
        nc.gpsimd.load_library(library_config.mlp)
        # First Q7-executed op after the reload faults the library blob in;
        # T gates the x DMAs so they don't contend with that load.
        nc.gpsimd.memset(tw[:], 0.0).then_inc(T, 1)
        nc.gpsimd.wait_ge(L, 16)
        # gather g covers slots/blocks [g*GBLK, (g+1)*GBLK) on SWDGE queue g
        # (queue q's desc-gen runs on Q7 core pair q -> 4-way parallel).
        for g in range(NGATH):
            nc.gpsimd.dma_gather(
                c_t[:, g * GBLK * D:(g + 1) * GBLK * D].rearrange(
                    "p (j d) -> p j d", d=D),
                centers[:],
                lab_t[:, g * GBLK * (P // 16):(g + 1) * GBLK * (P // 16)],
                GBLK * P, GBLK * P, D, queue_num=g,
            ).then_inc(G[g], 16)

        # ---- Scalar/ACT: x half 1 on its HWDGE ring, squares ----
        # A events: 1=xx h1, 2=xx h2, 3=mn h1, 4=mn h2, 4+i+1 = chunk i acc
        # Dummy sqrt (scale=0, bias=1 -> sqrt(1)) pulls the ACT table load
        # ahead; mn is rewritten by the real sqrts below.
        nc.scalar.activation(mn[:, :1], mn[:, :1],
                             mybir.ActivationFunctionType.Sqrt,
                             bias=1.0, scale=0.0)
        nc.scalar.wait_ge(T, 1)
        nc.scalar.dma_start(x_t[:, :H], xr[:, :H]).then_inc(X1, 16)
        nc.scalar.wait_ge(X1, 16)
        nc.scalar.square(xx[:, :H], x_t[:, :H]).then_inc(A, 1)
        nc.scalar.wait_ge(X2, 16)
        nc.scalar.square(xx[:, H:], x_t[:, H:]).then_inc(A, 1)
        nc.scalar.wait_ge(V, 1)
        nc.scalar.sqrt(mn[:, :J // 2], sx[:, :J // 2]).then_inc(A, 1)
        nc.scalar.wait_ge(V, 2)
        nc.scalar.sqrt(mn[:, J // 2:], sx[:, J // 2:]).then_inc(A, 1)
        for k in range(NGATH):
            f0, fn = k * GBLK * D, GBLK * D
            nc.scalar.wait_ge(V, 7 + k)
            nc.scalar.activation(c_t[:, f0:f0 + fn], c_t[:, f0:f0 + fn],
                                 mybir.ActivationFunctionType.Square,
                                 accum_out=acc[:, k:k + 1]).then_inc(A, 1)

        # ---- Vector/DVE ----
        # V events: 1=sx h1, 2=sx h2, 3=inv h1, 4=inv h2, 5=nx h1, 6=nx h2,
        # 6+i+1 = chunk i sub done
        def half(t, h):
            return t[:, h * H:(h + 1) * H].rearrange("p (j d) -> p j d", d=D)

        for h in range(2):
            nc.vector.wait_ge(A, 1 + h)
            nc.vector.reduce_sum(sx[:, h * J // 2:(h + 1) * J // 2],
                                 half(xx, h), axis=mybir.AxisListType.X
                                 ).then_inc(V, 1)
        for h in range(2):
            nc.vector.wait_ge(A, 3 + h)
            nc.vector.reciprocal(inv[:, h * J // 2:(h + 1) * J // 2],
                                 mn[:, h * J // 2:(h + 1) * J // 2]
                                 ).then_inc(V, 1)
        iap = inv[:]
        for h in range(2):
            nc.vector.wait_ge(V, 3 + h)
            ib = bass.AP(tensor=iap.tensor,
                         offset=iap.offset + h * (J // 2),
                         ap=[list(iap.ap[0]), [1, J // 2], [0, D]])
            nc.vector.tensor_tensor(out=half(nx, h), in0=half(x_t, h),
                                    in1=ib, op=mybir.AluOpType.mult
                                    ).then_inc(V, 1)
        nc.vector.wait_ge(V, 6)
        for k in range(NGATH):
            f0, fn = k * GBLK * D, GBLK * D
            nc.vector.wait_ge(G[k], 16)
            nc.vector.tensor_sub(c_t[:, f0:f0 + fn], nx[:, f0:f0 + fn],
                                 c_t[:, f0:f0 + fn]).then_inc(V, 1)

    nc.compile()
    return nc


def _get_nc():
    if "nc" not in _CACHE:
        _CACHE["nc"] = _build()
    return _CACHE["nc"]


def _prep_labels(lab_shard):
    """int16 idx layout for dma_gather: gather slot i = j*128+p must hold
    the label of x row p*16+j (so dst[i%128, i//128] aligns with x_t);
    then wrap slots into 16 partitions (idxs[c, s] = slot s*16+c) and
    replicate for the 8 Q7 core groups."""
    slots = lab_shard.reshape(P, J).T.reshape(-1)          # slot j*128+p
    wrapped = slots.reshape(ROWS // 16, 16).T              # [16, ROWS/16]
    return np.ascontiguousarray(
        np.tile(wrapped, (8, 1)).astype(np.int16))         # [128, ROWS/16]


def _run(x, labels, centers, trace=False):
    from concourse.bass_utils import run_bass_kernel_spmd

    x = np.ascontiguousarray(np.asarray(x, dtype=np.float32))
    labels = np.asarray(labels).astype(np.int16)
    centers = np.ascontiguousarray(np.asarray(centers, dtype=np.float32))

    in_maps = []
    for i in range(N_CORES):
        in_maps.append({
            "x": x[i * ROWS:(i + 1) * ROWS],
            "labels": _prep_labels(labels[i * ROWS:(i + 1) * ROWS]),
            "centers": centers,
        })
    res = run_bass_kernel_spmd(_get_nc(), in_maps,
                               core_ids=list(range(N_CORES)), trace=trace)
    total = np.float64(0.0)
    for r in res.results:
        total += np.float64(r["out"].sum(dtype=np.float64))
    loss = np.array(np.float32(total / B))
    return loss, res


def kernel(x, labels, centers):
    loss, _ = _run(x, labels, centers, trace=False)
    return loss


# revision 17
# speedup vs baseline: 1.0552x; 1.0024x over previous
"""CenterLoss kernel for Trainium2 (8 NeuronCores, data-parallel over batch).

loss = mean_i( ||nx_i||^2 + ||c_{l_i}||^2 - 2*nx_i.c_{l_i} )
     = mean_i( ||nx_i - c_{l_i}||^2 ),  nx_i = x_i / max(||x_i||, EPS)

The reference's (batch, num_classes) distmat is masked down to one column
per row, so only a gather of centers[labels] is needed (memory regime).

Sharding: batch 16384 -> 8 cores x 2048 rows, centers replicated. x row
p*16+j of a core's shard lives at SBUF partition p, free block j.
The centers gather uses InstDMAGatherAnt (gpsimd dma_gather), which is
Q7 descriptor-generation bound (~8.5ns/row on one tx/rx core pair; a
hot-labels probe showed no DRAM-locality sensitivity). Queue q's
desc-gen runs on Q7 core pair q (the plain indirect-DMA path is frozen
to pair 0), so the gather is split across all 4 SWDGE queues; HW
confirmed the pairs run ahead and overlap. The mlp library's ~6us IRAM
load is triggered by the FIRST Q7-executed op after the reload and
contends with concurrent HBM DMAs (10us when racing the x load), so a
tiny memset right after the reload triggers it immediately and signals
T; the x DMA waits for T and is split across the sync+scalar HWDGE
rings with the normalize pipeline run per half. Gather slot i writes
dst[i%128, i//128]; labels are permuted host-side (slot j*128+p =
label of x row p*16+j) and pre-wrapped into the int16 [16, n/16]
layout (replicated to all 8 Q7 core groups). Each gather has its own
semaphore (a shared sem races: its value can mix two gathers'
per-engine completions). A dummy sqrt preloads the ACT table. d = nx-c
and the Square+accumulate reduction are chunked per gather. Raw bacc
with manual semaphores. Each core returns per-partition partial sums;
the host combines.
"""

import numpy as np

B, C, D = 16384, 8192, 64
N_CORES = 8
ROWS = B // N_CORES        # 2048
P = 128
J = ROWS // P              # 16 blocks of D per partition
F = J * D                  # 1024 f32 per partition
NGATH = 4                  # one gather per SWDGE queue / Q7 pair
GBLK = J // NGATH          # 4 J-blocks per gather
H = F // 2                 # x pipeline half size

_CACHE = {}


def _build():
    from contextlib import ExitStack

    import concourse.bass as bass
    from concourse import bacc, library_config, mybir

    nc = bacc.Bacc("TRN2", target_bir_lowering=False, debug=False,
                   num_devices=N_CORES, dynamic_dma_scratch_size=131072,
                   num_swdge_queues=NGATH)
    f32 = mybir.dt.float32
    x = nc.dram_tensor("x", [ROWS, D], f32, kind="ExternalInput").ap()
    labels = nc.dram_tensor("labels", [P, ROWS // 16], mybir.dt.int16,
                            kind="ExternalInput").ap()
    centers = nc.dram_tensor("centers", [C, D], f32,
                             kind="ExternalInput").ap()
    out = nc.dram_tensor("out", [P, NGATH], f32, kind="ExternalOutput").ap()

    with ExitStack() as ctx:
        def sb(n, s, dt=f32):
            return ctx.enter_context(nc.sbuf_tensor(n, s, dt))
        lab_t = sb("lab_t", [P, ROWS // 16], mybir.dt.int16)
        tw = sb("tw", [P, 1])
        x_t = sb("x_t", [P, F])
        c_t = sb("c_t", [P, F])
        xx = sb("xx", [P, F])
        sx = sb("sx", [P, J])
        mn = sb("mn", [P, J])
        inv = sb("inv", [P, J])
        nx = sb("nx", [P, F])
        acc = sb("acc", [P, NGATH])
        L = ctx.enter_context(nc.semaphore("Lsem"))
        T = ctx.enter_context(nc.semaphore("Tsem"))
        X1 = ctx.enter_context(nc.semaphore("X1sem"))
        X2 = ctx.enter_context(nc.semaphore("X2sem"))
        G = [ctx.enter_context(nc.semaphore(f"Gsem{g}")) for g in range(NGATH)]
        A = ctx.enter_context(nc.semaphore("Asem"))   # ACT-produced events
        V = ctx.enter_context(nc.semaphore("Vsem"))   # DVE-produced events

        xr = x.rearrange("(p j) d -> p (j d)", p=P)

        # ---- Sync: labels in, x half 2 (after IRAM load), result out ----
        nc.sync.dma_start(lab_t[:], labels[:]).then_inc(L, 16)
        nc.sync.wait_ge(T, 1)
        nc.sync.dma_start(x_t[:, H:], xr[:, H:]).then_inc(X2, 16)
        nc.sync.wait_ge(A, 4 + NGATH)
        # No final wait on the out DMA: the bacc epilogue's engine drain
        # covers its completion, overlapping the exit barrier ladder.
        nc.sync.dma_start(out, acc[:]).then_inc(L, 16)

        # ---- GpSimd: trigger the mlp IRAM load, then the gathers ----
        nc.gpsimd.load_library(library_config.mlp)
        # First Q7-executed op after the reload faults the library blob in;
        # T gates the x DMAs so they don't contend with that load.
        nc.gpsimd.memset(tw[:], 0.0).then_inc(T, 1)
        nc.gpsimd.wait_ge(L, 16)
        # gather g covers slots/blocks [g*GBLK, (g+1)*GBLK) on SWDGE queue g
        # (queue q's desc-gen runs on Q7 core pair q -> 4-way parallel).
        for g in range(NGATH):
            nc.gpsimd.dma_gather(
                c_t[:, g * GBLK * D:(g + 1) * GBLK * D].rearrange(
                    "p (j d) -> p j d", d=D),
                centers[:],
                lab_t[:, g * GBLK * (P // 16):(g + 1) * GBLK * (P // 16)],
                GBLK * P, GBLK * P, D, queue_num=g,
            ).then_inc(G[g], 16)

        # ---- Scalar/ACT: x half 1 on its HWDGE ring, squares ----
        # A events: 1=xx h1, 2=xx h2, 3=mn h1, 4=mn h2, 4+i+1 = chunk i acc
        # ACT table load + dummy sqrt sit AFTER the T gate: their table DMAs
        # would otherwise contend with the mlp IRAM load (6.2us clean vs
        # 8.6us contended); the Sqrt table is only needed ~5us after T.
        nc.scalar.wait_ge(T, 1)
        nc.scalar.dma_start(x_t[:, :H], xr[:, :H]).then_inc(X1, 16)
        # Dummy sqrt (scale=0, bias=1 -> sqrt(1)) pulls the ACT table load
        # ahead of the real sqrts; mn is rewritten below.
        nc.scalar.activation(mn[:, :1], mn[:, :1],
                             mybir.ActivationFunctionType.Sqrt,
                             bias=1.0, scale=0.0)
        nc.scalar.wait_ge(X1, 16)
        nc.scalar.square(xx[:, :H], x_t[:, :H]).then_inc(A, 1)
        nc.scalar.wait_ge(X2, 16)
        nc.scalar.square(xx[:, H:], x_t[:, H:]).then_inc(A, 1)
        nc.scalar.wait_ge(V, 1)
        nc.scalar.sqrt(mn[:, :J // 2], sx[:, :J // 2]).then_inc(A, 1)
        nc.scalar.wait_ge(V, 2)
        nc.scalar.sqrt(mn[:, J // 2:], sx[:, J // 2:]).then_inc(A, 1)
        for k in range(NGATH):
            f0, fn = k * GBLK * D, GBLK * D
            nc.scalar.wait_ge(V, 7 + k)
            nc.scalar.activation(c_t[:, f0:f0 + fn], c_t[:, f0:f0 + fn],
                                 mybir.ActivationFunctionType.Square,
                                 accum_out=acc[:, k:k + 1]).then_inc(A, 1)

        # ---- Vector/DVE ----
        # V events: 1=sx h1, 2=sx h2, 3=inv h1, 4=inv h2, 5=nx h1, 6=nx h2,
        # 6+i+1 = chunk i sub done
        def half(t, h):
            return t[:, h * H:(h + 1) * H].rearrange("p (j d) -> p j d", d=D)

        for h in range(2):
            nc.vector.wait_ge(A, 1 + h)
            nc.vector.reduce_sum(sx[:, h * J // 2:(h + 1) * J // 2],
                                 half(xx, h), axis=mybir.AxisListType.X
                                 ).then_inc(V, 1)
        for h in range(2):
            nc.vector.wait_ge(A, 3 + h)
            nc.vector.reciprocal(inv[:, h * J // 2:(h + 1) * J // 2],
                                 mn[:, h * J // 2:(h + 1) * J // 2]
                                 ).then_inc(V, 1)
        iap = inv[:]
        for h in range(2):
            nc.vector.wait_ge(V, 3 + h)
            ib = bass.AP(tensor=iap.tensor,
                         offset=iap.offset + h * (J // 2),
                         ap=[list(iap.ap[0]), [1, J // 2], [0, D]])
            nc.vector.tensor_tensor(out=half(nx, h), in0=half(x_t, h),
                                    in1=ib, op=mybir.AluOpType.mult
                                    ).then_inc(V, 1)
        nc.vector.wait_ge(V, 6)
        for k in range(NGATH):
            f0, fn = k * GBLK * D, GBLK * D
            nc.vector.wait_ge(G[k], 16)
            nc.vector.tensor_sub(c_t[:, f0:f0 + fn], nx[:, f0:f0 + fn],
                                 c_t[:, f0:f0 + fn]).then_inc(V, 1)

    nc.compile()
    return nc


def _get_nc():
    if "nc" not in _CACHE:
        _CACHE["nc"] = _build()
    return _CACHE["nc"]


def _prep_labels(lab_shard):
    """int16 idx layout for dma_gather: gather slot i = j*128+p must hold
    the label of x row p*16+j (so dst[i%128, i//128] aligns with x_t);
    then wrap slots into 16 partitions (idxs[c, s] = slot s*16+c) and
    replicate for the 8 Q7 core groups."""
    slots = lab_shard.reshape(P, J).T.reshape(-1)          # slot j*128+p
    wrapped = slots.reshape(ROWS // 16, 16).T              # [16, ROWS/16]
    return np.ascontiguousarray(
        np.tile(wrapped, (8, 1)).astype(np.int16))         # [128, ROWS/16]


def _run(x, labels, centers, trace=False):
    from concourse.bass_utils import run_bass_kernel_spmd

    x = np.ascontiguousarray(np.asarray(x, dtype=np.float32))
    labels = np.asarray(labels).astype(np.int16)
    centers = np.ascontiguousarray(np.asarray(centers, dtype=np.float32))

    in_maps = []
    for i in range(N_CORES):
        in_maps.append({
            "x": x[i * ROWS:(i + 1) * ROWS],
            "labels": _prep_labels(labels[i * ROWS:(i + 1) * ROWS]),
            "centers": centers,
        })
    res = run_bass_kernel_spmd(_get_nc(), in_maps,
                               core_ids=list(range(N_CORES)), trace=trace)
    total = np.float64(0.0)
    for r in res.results:
        total += np.float64(r["out"].sum(dtype=np.float64))
    loss = np.array(np.float32(total / B))
    return loss, res


def kernel(x, labels, centers):
    loss, _ = _run(x, labels, centers, trace=False)
    return loss
